# revision 1
# baseline (speedup 1.0000x reference)
"""Swin-style HST block pair (windowed attn + shifted windowed attn, each with MLP).

Self-contained kernel: takes FULL unsharded inputs, distributes data-parallel
over batch B=16 across the 8 NeuronCores (2 images per core) via jax.pmap,
returns the FULL output. Weights are broadcast to every core.
"""

import numpy as np
import jax
import jax.numpy as jnp

B, H, W, C = 16, 56, 56, 384
HEADS, WIN, SHIFT = 12, 7, 3
N = WIN * WIN
MLP_DIM = 4 * C
EPS = 1e-5
NW = (H // WIN) * (W // WIN)  # windows per image


def _rel_idx():
    coords = np.stack(
        np.meshgrid(np.arange(WIN), np.arange(WIN), indexing="ij")
    ).reshape(2, -1)
    rel = (coords[:, :, None] - coords[:, None, :]).transpose(1, 2, 0)
    rel[:, :, 0] += WIN - 1
    rel[:, :, 1] += WIN - 1
    rel[:, :, 0] *= 2 * WIN - 1
    return rel.sum(-1)  # [N, N]


REL_IDX = _rel_idx()


def _shift_mask():
    img = np.zeros((H, W), np.float32)
    cnt = 0
    for hs in (slice(0, -WIN), slice(-WIN, -SHIFT), slice(-SHIFT, None)):
        for ws in (slice(0, -WIN), slice(-WIN, -SHIFT), slice(-SHIFT, None)):
            img[hs, ws] = cnt
            cnt += 1
    mw = img.reshape(H // WIN, WIN, W // WIN, WIN).transpose(0, 2, 1, 3).reshape(-1, N)
    diff = mw[:, None, :] - mw[:, :, None]
    return np.where(diff != 0, -100.0, 0.0).astype(np.float32)  # [NW, N, N]


MASK_NP = _shift_mask()


def _win_part(x, b):  # [b,H,W,C] -> [b*NW, N, C]
    x = x.reshape(b, H // WIN, WIN, W // WIN, WIN, C).transpose(0, 1, 3, 2, 4, 5)
    return x.reshape(-1, N, C)


def _win_rev(w, b):  # [b*NW, N, C] -> [b,H,W,C]
    x = w.reshape(b, H // WIN, W // WIN, WIN, WIN, C).transpose(0, 1, 3, 2, 4, 5)
    return x.reshape(b, H, W, C)


def _ln(x, g, b):
    m = x.mean(-1, keepdims=True)
    v = ((x - m) ** 2).mean(-1, keepdims=True)
    return (x - m) / jnp.sqrt(v + EPS) * g + b


def _attn(xw, qkv_w, qkv_b, proj_w, proj_b, bias_hnn, mask):
    # xw: [nWB, N, C]; bias_hnn: [HEADS, N, N] (pre-gathered on host)
    nWB = xw.shape[0]
    d = C // HEADS
    qkv = (xw @ qkv_w + qkv_b).reshape(nWB, N, 3, HEADS, d).transpose(2, 0, 3, 1, 4)
    q, k, v = qkv[0] * (d**-0.5), qkv[1], qkv[2]
    a = jnp.einsum("whnd,whmd->whnm", q, k)
    a = a + bias_hnn[None]
    if mask is not None:
        nW = mask.shape[0]
        a = (a.reshape(nWB // nW, nW, HEADS, N, N) + mask[None, :, None]).reshape(
            nWB, HEADS, N, N
        )
    a = jax.nn.softmax(a, axis=-1)
    out = jnp.einsum("whnm,whmd->whnd", a, v).transpose(0, 2, 1, 3).reshape(nWB, N, C)
    return out @ proj_w + proj_b


def _mlp(x, w1, b1, w2, b2):
    return jax.nn.gelu(x @ w1 + b1, approximate=False) @ w2 + b2


def _block_pair(
    x,
    ln1_g, ln1_b, qkv1_w, qkv1_b, proj1_w, proj1_b, bias1_hnn,
    ln2_g, ln2_b, mlp1_w1, mlp1_b1, mlp1_w2, mlp1_b2,
    ln3_g, ln3_b, qkv2_w, qkv2_b, proj2_w, proj2_b, bias2_hnn,
    ln4_g, ln4_b, mlp2_w1, mlp2_b1, mlp2_w2, mlp2_b2,
):
    # x: [b_local, H*W, C] on one core
    b = x.shape[0]
    mask = jnp.asarray(MASK_NP)
    # block 1: plain windowed attention
    h = _ln(x, ln1_g, ln1_b).reshape(b, H, W, C)
    h = _attn(_win_part(h, b), qkv1_w, qkv1_b, proj1_w, proj1_b, bias1_hnn, None)
    x = x + _win_rev(h, b).reshape(b, H * W, C)
    x = x + _mlp(_ln(x, ln2_g, ln2_b), mlp1_w1, mlp1_b1, mlp1_w2, mlp1_b2)
    # block 2: shifted windowed attention with mask
    h = _ln(x, ln3_g, ln3_b).reshape(b, H, W, C)
    h = jnp.roll(h, (-SHIFT, -SHIFT), axis=(1, 2))
    h = _attn(_win_part(h, b), qkv2_w, qkv2_b, proj2_w, proj2_b, bias2_hnn, mask)
    h = jnp.roll(_win_rev(h, b), (SHIFT, SHIFT), axis=(1, 2)).reshape(b, H * W, C)
    x = x + h
    x = x + _mlp(_ln(x, ln4_g, ln4_b), mlp2_w1, mlp2_b1, mlp2_w2, mlp2_b2)
    return x


_N_CORES = 8
_pmapped = jax.pmap(_block_pair, axis_name="b", in_axes=(0,) + (None,) * 26)


def kernel(
    x,
    ln1_g, ln1_b, qkv1_w, qkv1_b, proj1_w, proj1_b, bias1,
    ln2_g, ln2_b, mlp1_w1, mlp1_b1, mlp1_w2, mlp1_b2,
    ln3_g, ln3_b, qkv2_w, qkv2_b, proj2_w, proj2_b, bias2,
    ln4_g, ln4_b, mlp2_w1, mlp2_b1, mlp2_w2, mlp2_b2,
):
    x = np.asarray(x, dtype=np.float32)
    # pre-gather relative-position bias tables on host: [table, HEADS] -> [HEADS, N, N]
    b1 = np.asarray(bias1)[REL_IDX.reshape(-1)].reshape(N, N, HEADS)
    b1 = np.ascontiguousarray(b1.transpose(2, 0, 1), dtype=np.float32)
    b2 = np.asarray(bias2)[REL_IDX.reshape(-1)].reshape(N, N, HEADS)
    b2 = np.ascontiguousarray(b2.transpose(2, 0, 1), dtype=np.float32)

    xs = x.reshape(_N_CORES, B // _N_CORES, H * W, C)
    out = _pmapped(
        xs,
        np.asarray(ln1_g), np.asarray(ln1_b), np.asarray(qkv1_w), np.asarray(qkv1_b),
        np.asarray(proj1_w), np.asarray(proj1_b), b1,
        np.asarray(ln2_g), np.asarray(ln2_b), np.asarray(mlp1_w1), np.asarray(mlp1_b1),
        np.asarray(mlp1_w2), np.asarray(mlp1_b2),
        np.asarray(ln3_g), np.asarray(ln3_b), np.asarray(qkv2_w), np.asarray(qkv2_b),
        np.asarray(proj2_w), np.asarray(proj2_b), b2,
        np.asarray(ln4_g), np.asarray(ln4_b), np.asarray(mlp2_w1), np.asarray(mlp2_b1),
        np.asarray(mlp2_w2), np.asarray(mlp2_b2),
    )
    return np.asarray(out).reshape(B, H * W, C).astype(np.float32)


# revision 3
# speedup vs baseline: 1.7352x; 1.7352x over previous
"""Swin-style HST block pair (windowed attn + shifted windowed attn, each with MLP).

Self-contained kernel: takes FULL unsharded inputs, distributes data-parallel
over batch B=16 across the 8 NeuronCores (2 images per core) via jax.pmap,
returns the FULL output. Weights are broadcast to every core.
"""

import numpy as np
import jax
import jax.numpy as jnp

B, H, W, C = 16, 56, 56, 384
HEADS, WIN, SHIFT = 12, 7, 3
N = WIN * WIN
MLP_DIM = 4 * C
EPS = 1e-5
NW = (H // WIN) * (W // WIN)  # windows per image


def _rel_idx():
    coords = np.stack(
        np.meshgrid(np.arange(WIN), np.arange(WIN), indexing="ij")
    ).reshape(2, -1)
    rel = (coords[:, :, None] - coords[:, None, :]).transpose(1, 2, 0)
    rel[:, :, 0] += WIN - 1
    rel[:, :, 1] += WIN - 1
    rel[:, :, 0] *= 2 * WIN - 1
    return rel.sum(-1)  # [N, N]


REL_IDX = _rel_idx()


def _shift_mask():
    img = np.zeros((H, W), np.float32)
    cnt = 0
    for hs in (slice(0, -WIN), slice(-WIN, -SHIFT), slice(-SHIFT, None)):
        for ws in (slice(0, -WIN), slice(-WIN, -SHIFT), slice(-SHIFT, None)):
            img[hs, ws] = cnt
            cnt += 1
    mw = img.reshape(H // WIN, WIN, W // WIN, WIN).transpose(0, 2, 1, 3).reshape(-1, N)
    diff = mw[:, None, :] - mw[:, :, None]
    return np.where(diff != 0, -100.0, 0.0).astype(np.float32)  # [NW, N, N]


MASK_NP = _shift_mask()


def _win_part(x, b):  # [b,H,W,C] -> [b*NW, N, C]
    x = x.reshape(b, H // WIN, WIN, W // WIN, WIN, C).transpose(0, 1, 3, 2, 4, 5)
    return x.reshape(-1, N, C)


def _win_rev(w, b):  # [b*NW, N, C] -> [b,H,W,C]
    x = w.reshape(b, H // WIN, W // WIN, WIN, WIN, C).transpose(0, 1, 3, 2, 4, 5)
    return x.reshape(b, H, W, C)


def _ln(x, g, b):
    m = x.mean(-1, keepdims=True)
    v = ((x - m) ** 2).mean(-1, keepdims=True)
    return (x - m) / jnp.sqrt(v + EPS) * g + b


def _attn(xw, qkv_w, qkv_b, proj_w, proj_b, bias_hnn, mask):
    # xw: [nWB, N, C]; bias_hnn: [HEADS, N, N] (pre-gathered on host)
    nWB = xw.shape[0]
    d = C // HEADS
    qkv = (xw @ qkv_w + qkv_b).reshape(nWB, N, 3, HEADS, d).transpose(2, 0, 3, 1, 4)
    q, k, v = qkv[0] * (d**-0.5), qkv[1], qkv[2]
    a = jnp.einsum("whnd,whmd->whnm", q, k)
    a = a + bias_hnn[None]
    if mask is not None:
        nW = mask.shape[0]
        a = (a.reshape(nWB // nW, nW, HEADS, N, N) + mask[None, :, None]).reshape(
            nWB, HEADS, N, N
        )
    a = jax.nn.softmax(a, axis=-1)
    out = jnp.einsum("whnm,whmd->whnd", a, v).transpose(0, 2, 1, 3).reshape(nWB, N, C)
    return out @ proj_w + proj_b


def _mlp(x, w1, b1, w2, b2):
    return jax.nn.gelu(x @ w1 + b1, approximate=False) @ w2 + b2


def _block_pair(
    x,
    ln1_g, ln1_b, qkv1_w, qkv1_b, proj1_w, proj1_b, bias1_hnn,
    ln2_g, ln2_b, mlp1_w1, mlp1_b1, mlp1_w2, mlp1_b2,
    ln3_g, ln3_b, qkv2_w, qkv2_b, proj2_w, proj2_b, bias2_hnn,
    ln4_g, ln4_b, mlp2_w1, mlp2_b1, mlp2_w2, mlp2_b2,
):
    # x: [b_local, H*W, C] on one core
    b = x.shape[0]
    mask = jnp.asarray(MASK_NP)
    # block 1: plain windowed attention
    h = _ln(x, ln1_g, ln1_b).reshape(b, H, W, C)
    h = _attn(_win_part(h, b), qkv1_w, qkv1_b, proj1_w, proj1_b, bias1_hnn, None)
    x = x + _win_rev(h, b).reshape(b, H * W, C)
    x = x + _mlp(_ln(x, ln2_g, ln2_b), mlp1_w1, mlp1_b1, mlp1_w2, mlp1_b2)
    # block 2: shifted windowed attention with mask
    h = _ln(x, ln3_g, ln3_b).reshape(b, H, W, C)
    h = jnp.roll(h, (-SHIFT, -SHIFT), axis=(1, 2))
    h = _attn(_win_part(h, b), qkv2_w, qkv2_b, proj2_w, proj2_b, bias2_hnn, mask)
    h = jnp.roll(_win_rev(h, b), (SHIFT, SHIFT), axis=(1, 2)).reshape(b, H * W, C)
    x = x + h
    x = x + _mlp(_ln(x, ln4_g, ln4_b), mlp2_w1, mlp2_b1, mlp2_w2, mlp2_b2)
    return x


_N_CORES = 8
_pmapped = jax.pmap(_block_pair, axis_name="b", in_axes=(0,) * 27)

_weight_cache = {}


def _replicated(w):
    # cache device-replicated copies of the (static) weights so repeat calls
    # only transfer the activation tensor
    key = id(w)
    hit = _weight_cache.get(key)
    if hit is None:
        hit = jax.device_put_replicated(
            jnp.asarray(w), jax.devices()[:_N_CORES]
        )
        _weight_cache[key] = hit
    return hit


def kernel(
    x,
    ln1_g, ln1_b, qkv1_w, qkv1_b, proj1_w, proj1_b, bias1,
    ln2_g, ln2_b, mlp1_w1, mlp1_b1, mlp1_w2, mlp1_b2,
    ln3_g, ln3_b, qkv2_w, qkv2_b, proj2_w, proj2_b, bias2,
    ln4_g, ln4_b, mlp2_w1, mlp2_b1, mlp2_w2, mlp2_b2,
):
    x = np.asarray(x, dtype=np.float32)
    # pre-gather relative-position bias tables on host: [table, HEADS] -> [HEADS, N, N]
    b1 = np.asarray(bias1)[REL_IDX.reshape(-1)].reshape(N, N, HEADS)
    b1 = np.ascontiguousarray(b1.transpose(2, 0, 1), dtype=np.float32)
    b2 = np.asarray(bias2)[REL_IDX.reshape(-1)].reshape(N, N, HEADS)
    b2 = np.ascontiguousarray(b2.transpose(2, 0, 1), dtype=np.float32)

    xs = x.reshape(_N_CORES, B // _N_CORES, H * W, C)
    ws = [
        ln1_g, ln1_b, qkv1_w, qkv1_b, proj1_w, proj1_b, b1,
        ln2_g, ln2_b, mlp1_w1, mlp1_b1, mlp1_w2, mlp1_b2,
        ln3_g, ln3_b, qkv2_w, qkv2_b, proj2_w, proj2_b, b2,
        ln4_g, ln4_b, mlp2_w1, mlp2_b1, mlp2_w2, mlp2_b2,
    ]
    out = _pmapped(xs, *[_replicated(np.asarray(w)) for w in ws])
    return np.asarray(out).reshape(B, H * W, C).astype(np.float32)
